# revision 13
# baseline (speedup 1.0000x reference)
"""GNN message-passing kernel for Trainium2 (8 NeuronCores, batch-parallel).

Computation (per reference):
    norm_adj = adjacency * dinv * dinv.T + I            [10,10]   (host, O(100) flops)
    support  = einsum('bcf,fo->bco', x, kernel)         [B,C,O]
    out      = elu(einsum('ij,bjo->bio', norm_adj, support) + bias)
    out      = (out - mean) * rsqrt(var+eps) * gamma + beta

Device strategy per core (512 batches = 5120 rows of [b,c] x f), bf16 operands
(rel-err ~6e-3, well under the 2e-2 gate; bf16 runs the PE at full rate, halves
DMA traffic, and enables fast weight loads):
  1. "Transposing mix": PE matmul with x-chunks [crows<=120, 128f] as the
     stationary operand and a block-diagonal norm_adj [crows, crows] as the
     moving operand. One op applies the channel mix and lands the activations
     transposed ([f, rows]) as needed by the main matmul. The short mix MMs
     are interleaved between the long main-matmul o-tiles of the previous
     panel so their weight loads hide under the running main matmuls.
  2. Main matmul: outT[o,rows] += K[f,o].T @ yT[f,rows], kernel resident in
     SBUF (bf16). Mix chunks are decoupled from panels: a chunk whose rows
     straddle a panel boundary is copied into both panels' yT tiles.
  3. Epilogue on ACT/DVE with per-partition (o) params:
     elu(z) = min(exp(z), relu(z)+1) - 1 (exact), then folded BN affine.
     Output stored transposed [O, rows] bf16; host transposes/casts while
     unsharding. Output DMA rides the otherwise-idle Pool queue so the ACT
     queue only carries the activation ops.
"""

from contextlib import ExitStack

import numpy as np
import ml_dtypes

import concourse.bass as bass
import concourse.bacc as bacc
import concourse.mybir as mybir
import concourse.tile as tile
from concourse.bass_utils import run_bass_kernel_spmd

F32 = mybir.dt.float32
BF16 = mybir.dt.bfloat16
ALU = mybir.AluOpType
ACTF = mybir.ActivationFunctionType

P = 128
BN_EPS = 1e-3
N_CORES = 8
C = 10  # channels
CHUNKS = (12, 12, 8)  # batches per mix chunk (x10 rows each), cycled over rows
PANEL = 320  # main-matmul moving width (<=512: one fp32 PSUM bank)


def build_nc(rows, F, O, chunk_batches=CHUNKS, n_cores=N_CORES, repeats=1,
             panel=PANEL, xpool_bufs=6, mainps_bufs=4):
    """Build the per-core Bass program. rows = local (b,c) rows, F/O = feat dims."""
    assert rows % panel == 0
    n_panels = rows // panel
    FT, OT = F // P, O // P
    bd_sizes = sorted({nb * C for nb in chunk_batches})
    bd_off = {}
    off = 0
    for sz in bd_sizes:
        bd_off[sz] = off
        off += sz
    mixb_cols = off

    # chunk list: (start_row, n_rows) cycling through chunk_batches
    chunk_list = []
    r = 0
    ci = 0
    while r < rows:
        crows = chunk_batches[ci % len(chunk_batches)] * C
        assert r + crows <= rows
        chunk_list.append((r, crows))
        r += crows
        ci += 1

    nc = bacc.Bacc(
        "TRN2",
        target_bir_lowering=False,
        debug=False,
        enable_asserts=False,
        num_devices=n_cores,
    )
    x_d = nc.dram_tensor("x_local", [rows, F], BF16, kind="ExternalInput").ap()
    k_d = nc.dram_tensor("kern", [F, O], BF16, kind="ExternalInput").ap()
    mixb_d = nc.dram_tensor("mixb", [P, mixb_cols], BF16, kind="ExternalInput").ap()
    # prm cols [0:OT]=bias_t, [OT:2OT]=scale_t, [2OT:3OT]=shift2_t (per-partition o)
    prm_d = nc.dram_tensor("prm", [P, 3 * OT], F32, kind="ExternalInput").ap()
    outT_d = nc.dram_tensor("outT", [O, rows], BF16, kind="ExternalOutput").ap()

    with tile.TileContext(nc) as tc, ExitStack() as ctx:
        const = ctx.enter_context(tc.tile_pool(name="const", bufs=1))
        mixb = const.tile([P, mixb_cols], BF16, name="mixb")
        prm = const.tile([P, 3 * OT], F32, name="prm")
        nc.sync.dma_start(mixb, mixb_d)
        nc.sync.dma_start(prm, prm_d)
        bd_t = {sz: mixb[:sz, bd_off[sz] : bd_off[sz] + sz] for sz in bd_sizes}
        kb = [const.tile([P, O], BF16, name=f"kb{fb}", tag=f"kb{fb}") for fb in range(FT)]
        for fb in range(FT):
            nc.gpsimd.dma_start(kb[fb], k_d[fb * P : (fb + 1) * P, :])

        xpool = ctx.enter_context(tc.tile_pool(name="xpool", bufs=xpool_bufs))
        ypool = ctx.enter_context(tc.tile_pool(name="ypool", bufs=3))
        mixps = ctx.enter_context(tc.tile_pool(name="mixps", bufs=2, space="PSUM"))
        mainps = ctx.enter_context(tc.tile_pool(name="mainps", bufs=mainps_bufs, space="PSUM"))
        tmp = ctx.enter_context(tc.tile_pool(name="tmp", bufs=2))

        yts = {}

        def get_yt(rep, pi):
            if (rep, pi) not in yts:
                yts[(rep, pi)] = ypool.tile(
                    [P, FT, panel], BF16, name=f"r{rep}_yt_{pi}", tag="yt"
                )
            return yts[(rep, pi)]

        def mix_groups(rep, chunks):
            """Yield thunks: one per (chunk, 4-f-block) mix group (4 MMs + copies).
            The per-chunk x DMA is emitted with the chunk's first group."""
            for start, crows in chunks:
                def load(start=start, crows=crows):
                    xt = xpool.tile([120, F], BF16, name=f"r{rep}_x_{start}", tag="xc")[:crows]
                    nc.sync.dma_start(xt, x_d[start : start + crows, :])
                    return xt

                holder = {}
                for fbp in range(FT // 4):
                    def group(start=start, crows=crows, fbp=fbp, holder=holder, load=load):
                        if "xt" not in holder:
                            holder["xt"] = load()
                        xt = holder["xt"]
                        fb = 4 * fbp
                        ps = mixps.tile([P, 4, 120], F32,
                                        name=f"r{rep}_mps_{start}_{fbp}", tag="mixps")
                        for q in range(4):
                            nc.tensor.matmul(
                                ps[:, q, :crows],
                                lhsT=xt[:, (fb + q) * P : (fb + q + 1) * P],
                                rhs=bd_t[crows],
                                start=True,
                                stop=True,
                            )
                        # copy to the panel(s) this chunk covers
                        s, e = start, start + crows
                        p0, p1 = s // panel, (e - 1) // panel
                        for pp in range(p0, p1 + 1):
                            lo = max(s, pp * panel)
                            hi = min(e, (pp + 1) * panel)
                            nc.vector.tensor_copy(
                                get_yt(rep, pp)[:, fb : fb + 4, lo - pp * panel : hi - pp * panel],
                                ps[:, :, lo - s : hi - s],
                            )
                    yield group

        def emit_main_otile(rep, pi, ot):
            row0 = pi * panel
            ytall = yts[(rep, pi)]
            ps = mainps.tile([P, panel], F32, name=f"r{rep}_ops_{pi}_{ot}", tag="mainps")
            for fb in range(FT):
                nc.tensor.matmul(
                    ps,
                    lhsT=kb[fb][:, ot * P : (ot + 1) * P],
                    rhs=ytall[:, fb, :],
                    start=(fb == 0),
                    stop=(fb == FT - 1),
                )
            bias_ap = prm[:, ot : ot + 1]
            scale_ap = prm[:, OT + ot : OT + ot + 1]
            shift_ap = prm[:, 2 * OT + ot : 2 * OT + ot + 1]
            e = tmp.tile([P, panel], F32, name=f"r{rep}_e_{pi}_{ot}", tag="e")
            t0 = tmp.tile([P, panel], F32, name=f"r{rep}_t0_{pi}_{ot}", tag="t0")
            s = tmp.tile([P, panel], F32, name=f"r{rep}_s_{pi}_{ot}", tag="s")
            fin = tmp.tile([P, panel], BF16, name=f"r{rep}_fin_{pi}_{ot}", tag="fin")
            nc.scalar.activation(e, ps, ACTF.Exp, bias=bias_ap)
            nc.scalar.activation(t0, ps, ACTF.Relu, bias=bias_ap)
            # elu(zb) + 1 = min(exp(zb), relu(zb) + 1)   (exact identity)
            nc.vector.scalar_tensor_tensor(
                s, in0=t0, scalar=1.0, in1=e, op0=ALU.add, op1=ALU.min
            )
            # fin = s*scale + (shift - scale) = elu*scale + shift
            nc.vector.tensor_scalar(
                fin, s, scale_ap, shift_ap, op0=ALU.mult, op1=ALU.add
            )
            nc.gpsimd.dma_start(outT_d[ot * P : (ot + 1) * P, row0 : row0 + panel], fin)

        # Pipeline: chunks whose start lies in panel p's row-range are mixed
        # while panel p-1's main matmul runs, interleaved between its o-tiles.
        # One flat stream across repeats.
        slots = []  # (rep, pi, [chunks starting in that panel])
        for rep in range(repeats):
            for pi in range(n_panels):
                slots.append((rep, pi,
                              [c for c in chunk_list
                               if pi * panel <= c[0] < (pi + 1) * panel]))
        for idx in range(len(slots) + 1):
            if idx < len(slots):
                rep, pi, chunks = slots[idx]
                get_yt(rep, pi)
                groups = list(mix_groups(rep, chunks))
            else:
                groups = []
            if idx == 0:
                for g in groups:
                    g()
                continue
            rep0, pi0, _ = slots[idx - 1]
            gi = 0
            n_g = max(len(groups), 1)
            for ot in range(OT):
                emit_main_otile(rep0, pi0, ot)
                want = min(((ot + 1) * n_g) // OT, len(groups))
                while gi < want:
                    groups[gi]()
                    gi += 1
            yts.pop((rep0, pi0))
    nc.compile()
    return nc


def _host_prep(adjacency, kern, bias, gamma, beta, moving_mean, moving_var,
               chunk_batches=CHUNKS, O=2048):
    """Build the tiny derived inputs on the host: mixb (bf16) and prm (f32)."""
    A = np.asarray(adjacency, np.float32)
    deg = np.maximum(np.abs(A).sum(axis=1, keepdims=True), 1e-8)
    dinv = deg ** -0.5
    na = A * dinv * dinv.T + np.eye(C, dtype=np.float32)  # [10,10]

    bd_sizes = sorted({nb * C for nb in chunk_batches})
    OT = O // P
    mixb_cols = sum(bd_sizes)
    mixb = np.zeros((P, mixb_cols), np.float32)
    off = 0
    for sz in bd_sizes:
        nb = sz // C
        for g in range(nb):
            mixb[g * C : (g + 1) * C, off + g * C : off + (g + 1) * C] = na.T
        off += sz
    scale = np.asarray(gamma, np.float32) / np.sqrt(np.asarray(moving_var, np.float32) + BN_EPS)
    shift2 = np.asarray(beta, np.float32) - np.asarray(moving_mean, np.float32) * scale - scale
    prm = np.empty((P, 3 * OT), np.float32)
    prm[:, :OT] = np.asarray(bias, np.float32).reshape(OT, P).T
    prm[:, OT : 2 * OT] = scale.reshape(OT, P).T
    prm[:, 2 * OT :] = shift2.reshape(OT, P).T
    return mixb.astype(ml_dtypes.bfloat16), prm


def make_in_maps(x, adjacency, kernel, bias, gamma, beta, moving_mean, moving_var):
    B, C_, F = x.shape
    O = kernel.shape[1]
    bl = B // N_CORES
    rows = bl * C
    mixb, prm = _host_prep(adjacency, kernel, bias, gamma, beta, moving_mean,
                           moving_var, CHUNKS, O)
    kern_np = np.ascontiguousarray(np.asarray(kernel, np.float32).astype(ml_dtypes.bfloat16))
    x_bf = np.asarray(x, np.float32).astype(ml_dtypes.bfloat16)
    in_maps = []
    for c in range(N_CORES):
        in_maps.append({
            "x_local": np.ascontiguousarray(x_bf[c * bl : (c + 1) * bl].reshape(rows, F)),
            "kern": kern_np,
            "mixb": mixb,
            "prm": prm,
        })
    return in_maps


def unshard(outT_per_core, B, O):
    """outT_per_core: list/array of [O, rows] bf16 per core -> [B, C, O] f32."""
    bl = B // N_CORES
    rows = bl * C
    out = np.empty((B, C, O), np.float32)
    for c in range(N_CORES):
        outT = np.asarray(outT_per_core[c], dtype=np.float32)  # [O, rows]
        out[c * bl : (c + 1) * bl] = outT.T.reshape(bl, C, O)
    return out


def kernel(x, adjacency, kernel, bias, gamma, beta, moving_mean, moving_var):
    B, C_, F = x.shape
    O = kernel.shape[1]
    assert C_ == C
    assert B % N_CORES == 0
    bl = B // N_CORES
    rows = bl * C

    in_maps = make_in_maps(x, adjacency, kernel, bias, gamma, beta,
                           moving_mean, moving_var)
    nc = build_nc(rows, F, O, CHUNKS)
    res = run_bass_kernel_spmd(nc, in_maps, core_ids=list(range(N_CORES)), trace=False)
    return unshard([res.results[c]["outT"] for c in range(N_CORES)], B, O)


# revision 14
# speedup vs baseline: 1.3716x; 1.3716x over previous
"""GNN message-passing kernel for Trainium2 (8 NeuronCores, batch-parallel).

Computation (per reference):
    norm_adj = adjacency * dinv * dinv.T + I            [10,10]   (host, O(100) flops)
    support  = einsum('bcf,fo->bco', x, kernel)         [B,C,O]
    out      = elu(einsum('ij,bjo->bio', norm_adj, support) + bias)
    out      = (out - mean) * rsqrt(var+eps) * gamma + beta

Device strategy per core (512 batches = 5120 rows of [b,c] x f), bf16 operands
(rel-err ~6e-3, well under the 2e-2 gate; bf16 runs the PE at full rate, halves
DMA traffic, and enables fast weight loads):
  1. "Transposing mix": PE matmul with x-chunks [crows<=120, 128f] as the
     stationary operand and a block-diagonal norm_adj [crows, crows] as the
     moving operand. One op applies the channel mix and lands the activations
     transposed ([f, rows]) as needed by the main matmul. The short mix MMs
     are interleaved between the long main-matmul o-tiles of the previous
     panel so their weight loads hide under the running main matmuls.
  2. Main matmul: outT[o,rows] += K[f,o].T @ yT[f,rows], kernel resident in
     SBUF (bf16). Mix chunks are decoupled from panels: a chunk whose rows
     straddle a panel boundary is copied into both panels' yT tiles.
  3. Epilogue on ACT/DVE with per-partition (o) params:
     elu(z) = min(exp(z), relu(z)+1) - 1 (exact), then folded BN affine.
     Output stored transposed [O, rows] bf16; host transposes/casts while
     unsharding. Output DMA rides the otherwise-idle Pool queue so the ACT
     queue only carries the activation ops.
"""

from contextlib import ExitStack

import numpy as np
import ml_dtypes

import concourse.bass as bass
import concourse.bacc as bacc
import concourse.mybir as mybir
import concourse.tile as tile
from concourse.bass_utils import run_bass_kernel_spmd

F32 = mybir.dt.float32
BF16 = mybir.dt.bfloat16
ALU = mybir.AluOpType
ACTF = mybir.ActivationFunctionType

P = 128
BN_EPS = 1e-3
N_CORES = 8
C = 10  # channels
CHUNKS = (12,)  # batches per mix chunk (x10 rows each), cycled; tail clamped
PANEL = 320  # main-matmul moving width (<=512: one fp32 PSUM bank)


def _chunk_rows(rows, chunk_batches):
    out = []
    r = 0
    i = 0
    while r < rows:
        crows = min(chunk_batches[i % len(chunk_batches)] * C, rows - r)
        out.append(crows)
        r += crows
        i += 1
    return out


def build_nc(rows, F, O, chunk_batches=CHUNKS, n_cores=N_CORES, repeats=1,
             panel=PANEL, xpool_bufs=6, mainps_bufs=4):
    """Build the per-core Bass program. rows = local (b,c) rows, F/O = feat dims."""
    assert rows % panel == 0
    n_panels = rows // panel
    FT, OT = F // P, O // P
    bd_sizes = sorted(set(_chunk_rows(rows, chunk_batches)))
    bd_off = {}
    off = 0
    for sz in bd_sizes:
        bd_off[sz] = off
        off += sz
    mixb_cols = off

    # chunk list: (start_row, n_rows) cycling through chunk_batches, with the
    # final chunk clamped to the remaining rows (still a whole-batch multiple)
    chunk_list = []
    r = 0
    ci = 0
    while r < rows:
        crows = min(chunk_batches[ci % len(chunk_batches)] * C, rows - r)
        assert crows % C == 0 and crows > 0
        chunk_list.append((r, crows))
        r += crows
        ci += 1

    nc = bacc.Bacc(
        "TRN2",
        target_bir_lowering=False,
        debug=False,
        enable_asserts=False,
        num_devices=n_cores,
    )
    x_d = nc.dram_tensor("x_local", [rows, F], BF16, kind="ExternalInput").ap()
    k_d = nc.dram_tensor("kern", [F, O], BF16, kind="ExternalInput").ap()
    mixb_d = nc.dram_tensor("mixb", [P, mixb_cols], BF16, kind="ExternalInput").ap()
    # prm cols [0:OT]=bias_t, [OT:2OT]=scale_t, [2OT:3OT]=shift2_t (per-partition o)
    prm_d = nc.dram_tensor("prm", [P, 3 * OT], F32, kind="ExternalInput").ap()
    outT_d = nc.dram_tensor("outT", [O, rows], BF16, kind="ExternalOutput").ap()

    with tile.TileContext(nc) as tc, ExitStack() as ctx:
        const = ctx.enter_context(tc.tile_pool(name="const", bufs=1))
        mixb = const.tile([P, mixb_cols], BF16, name="mixb")
        prm = const.tile([P, 3 * OT], F32, name="prm")
        nc.sync.dma_start(mixb, mixb_d)
        nc.sync.dma_start(prm, prm_d)
        bd_t = {sz: mixb[:sz, bd_off[sz] : bd_off[sz] + sz] for sz in bd_sizes}
        kb = [const.tile([P, O], BF16, name=f"kb{fb}", tag=f"kb{fb}") for fb in range(FT)]
        for fb in range(FT):
            nc.gpsimd.dma_start(kb[fb], k_d[fb * P : (fb + 1) * P, :])

        xpool = ctx.enter_context(tc.tile_pool(name="xpool", bufs=xpool_bufs))
        ypool = ctx.enter_context(tc.tile_pool(name="ypool", bufs=3))
        mixps = ctx.enter_context(tc.tile_pool(name="mixps", bufs=2, space="PSUM"))
        mainps = ctx.enter_context(tc.tile_pool(name="mainps", bufs=mainps_bufs, space="PSUM"))
        tmp = ctx.enter_context(tc.tile_pool(name="tmp", bufs=2))

        yts = {}

        def get_yt(rep, pi):
            if (rep, pi) not in yts:
                yts[(rep, pi)] = ypool.tile(
                    [P, FT, panel], BF16, name=f"r{rep}_yt_{pi}", tag="yt"
                )
            return yts[(rep, pi)]

        def mix_groups(rep, chunks):
            """Yield thunks: one per (chunk, 4-f-block) mix group (4 MMs + copies).
            The per-chunk x DMA is emitted with the chunk's first group."""
            for start, crows in chunks:
                def load(start=start, crows=crows):
                    xt = xpool.tile([120, F], BF16, name=f"r{rep}_x_{start}", tag="xc")[:crows]
                    nc.sync.dma_start(xt, x_d[start : start + crows, :])
                    return xt

                holder = {}
                for fbp in range(FT // 4):
                    def group(start=start, crows=crows, fbp=fbp, holder=holder, load=load):
                        if "xt" not in holder:
                            holder["xt"] = load()
                        xt = holder["xt"]
                        fb = 4 * fbp
                        ps = mixps.tile([P, 4, 120], F32,
                                        name=f"r{rep}_mps_{start}_{fbp}", tag="mixps")
                        for q in range(4):
                            nc.tensor.matmul(
                                ps[:, q, :crows],
                                lhsT=xt[:, (fb + q) * P : (fb + q + 1) * P],
                                rhs=bd_t[crows],
                                start=True,
                                stop=True,
                            )
                        # copy to the panel(s) this chunk covers
                        s, e = start, start + crows
                        p0, p1 = s // panel, (e - 1) // panel
                        for pp in range(p0, p1 + 1):
                            lo = max(s, pp * panel)
                            hi = min(e, (pp + 1) * panel)
                            nc.vector.tensor_copy(
                                get_yt(rep, pp)[:, fb : fb + 4, lo - pp * panel : hi - pp * panel],
                                ps[:, :, lo - s : hi - s],
                            )
                    yield group

        def emit_main_otile(rep, pi, ot):
            row0 = pi * panel
            ytall = yts[(rep, pi)]
            ps = mainps.tile([P, panel], F32, name=f"r{rep}_ops_{pi}_{ot}", tag="mainps")
            for fb in range(FT):
                nc.tensor.matmul(
                    ps,
                    lhsT=kb[fb][:, ot * P : (ot + 1) * P],
                    rhs=ytall[:, fb, :],
                    start=(fb == 0),
                    stop=(fb == FT - 1),
                )
            bias_ap = prm[:, ot : ot + 1]
            scale_ap = prm[:, OT + ot : OT + ot + 1]
            shift_ap = prm[:, 2 * OT + ot : 2 * OT + ot + 1]
            e = tmp.tile([P, panel], F32, name=f"r{rep}_e_{pi}_{ot}", tag="e")
            t0 = tmp.tile([P, panel], F32, name=f"r{rep}_t0_{pi}_{ot}", tag="t0")
            s = tmp.tile([P, panel], F32, name=f"r{rep}_s_{pi}_{ot}", tag="s")
            fin = tmp.tile([P, panel], BF16, name=f"r{rep}_fin_{pi}_{ot}", tag="fin")
            nc.scalar.activation(e, ps, ACTF.Exp, bias=bias_ap)
            nc.scalar.activation(t0, ps, ACTF.Relu, bias=bias_ap)
            # elu(zb) + 1 = min(exp(zb), relu(zb) + 1)   (exact identity)
            nc.vector.scalar_tensor_tensor(
                s, in0=t0, scalar=1.0, in1=e, op0=ALU.add, op1=ALU.min
            )
            # fin = s*scale + (shift - scale) = elu*scale + shift
            nc.vector.tensor_scalar(
                fin, s, scale_ap, shift_ap, op0=ALU.mult, op1=ALU.add
            )
            nc.gpsimd.dma_start(outT_d[ot * P : (ot + 1) * P, row0 : row0 + panel], fin)

        # Pipeline: chunks whose start lies in panel p's row-range are mixed
        # while panel p-1's main matmul runs, interleaved between its o-tiles.
        # One flat stream across repeats.
        slots = []  # (rep, pi, [chunks starting in that panel])
        for rep in range(repeats):
            for pi in range(n_panels):
                slots.append((rep, pi,
                              [c for c in chunk_list
                               if pi * panel <= c[0] < (pi + 1) * panel]))
        for idx in range(len(slots) + 1):
            if idx < len(slots):
                rep, pi, chunks = slots[idx]
                get_yt(rep, pi)
                groups = list(mix_groups(rep, chunks))
            else:
                groups = []
            if idx == 0:
                for g in groups:
                    g()
                continue
            rep0, pi0, _ = slots[idx - 1]
            gi = 0
            n_g = max(len(groups), 1)
            for ot in range(OT):
                emit_main_otile(rep0, pi0, ot)
                want = min(((ot + 1) * n_g) // OT, len(groups))
                while gi < want:
                    groups[gi]()
                    gi += 1
            yts.pop((rep0, pi0))
    nc.compile()
    return nc


def _host_prep(adjacency, kern, bias, gamma, beta, moving_mean, moving_var,
               chunk_batches=CHUNKS, O=2048, rows=5120):
    """Build the tiny derived inputs on the host: mixb (bf16) and prm (f32)."""
    A = np.asarray(adjacency, np.float32)
    deg = np.maximum(np.abs(A).sum(axis=1, keepdims=True), 1e-8)
    dinv = deg ** -0.5
    na = A * dinv * dinv.T + np.eye(C, dtype=np.float32)  # [10,10]

    bd_sizes = sorted(set(_chunk_rows(rows, chunk_batches)))
    OT = O // P
    mixb_cols = sum(bd_sizes)
    mixb = np.zeros((P, mixb_cols), np.float32)
    off = 0
    for sz in bd_sizes:
        nb = sz // C
        for g in range(nb):
            mixb[g * C : (g + 1) * C, off + g * C : off + (g + 1) * C] = na.T
        off += sz
    scale = np.asarray(gamma, np.float32) / np.sqrt(np.asarray(moving_var, np.float32) + BN_EPS)
    shift2 = np.asarray(beta, np.float32) - np.asarray(moving_mean, np.float32) * scale - scale
    prm = np.empty((P, 3 * OT), np.float32)
    prm[:, :OT] = np.asarray(bias, np.float32).reshape(OT, P).T
    prm[:, OT : 2 * OT] = scale.reshape(OT, P).T
    prm[:, 2 * OT :] = shift2.reshape(OT, P).T
    return mixb.astype(ml_dtypes.bfloat16), prm


def make_in_maps(x, adjacency, kernel, bias, gamma, beta, moving_mean, moving_var):
    B, C_, F = x.shape
    O = kernel.shape[1]
    bl = B // N_CORES
    rows = bl * C
    mixb, prm = _host_prep(adjacency, kernel, bias, gamma, beta, moving_mean,
                           moving_var, CHUNKS, O, rows)
    kern_np = np.ascontiguousarray(np.asarray(kernel, np.float32).astype(ml_dtypes.bfloat16))
    x_bf = np.asarray(x, np.float32).astype(ml_dtypes.bfloat16)
    in_maps = []
    for c in range(N_CORES):
        in_maps.append({
            "x_local": np.ascontiguousarray(x_bf[c * bl : (c + 1) * bl].reshape(rows, F)),
            "kern": kern_np,
            "mixb": mixb,
            "prm": prm,
        })
    return in_maps


def unshard(outT_per_core, B, O):
    """outT_per_core: list/array of [O, rows] bf16 per core -> [B, C, O] f32."""
    bl = B // N_CORES
    rows = bl * C
    out = np.empty((B, C, O), np.float32)
    for c in range(N_CORES):
        outT = np.asarray(outT_per_core[c], dtype=np.float32)  # [O, rows]
        out[c * bl : (c + 1) * bl] = outT.T.reshape(bl, C, O)
    return out


def kernel(x, adjacency, kernel, bias, gamma, beta, moving_mean, moving_var):
    B, C_, F = x.shape
    O = kernel.shape[1]
    assert C_ == C
    assert B % N_CORES == 0
    bl = B // N_CORES
    rows = bl * C

    in_maps = make_in_maps(x, adjacency, kernel, bias, gamma, beta,
                           moving_mean, moving_var)
    nc = build_nc(rows, F, O, CHUNKS)
    res = run_bass_kernel_spmd(nc, in_maps, core_ids=list(range(N_CORES)), trace=False)
    return unshard([res.results[c]["outT"] for c in range(N_CORES)], B, O)
